# revision 48
# baseline (speedup 1.0000x reference)
"""Trainium2 Bass kernel for GPT-NeoX-style attention block (nn_Attention_88141318848873).

Full inputs -> head-parallel tensor-parallel across 8 NeuronCores -> full output.

Per core c (local heads = global heads 4c..4c+3):
  - QKV projection in natural [tok, dim] layout (bf16 matmuls, fp32 PSUM accum),
    partial RoPE applied with free-dim strided DVE ops, then PE-transpose of q/k
    into [dim, tok] layout for the scores matmuls.
  - Scores computed transposed (S^T[k, q]) with two heads packed into the 128x128
    PE array via partition bases 0/64 (K=64 row tiling).
  - Softmax: no max subtraction (scores bounded ~|5| for this problem's scale),
    exp on ScalarE with causal block skipping, probabilities kept unnormalized.
  - PV flash-style: lhsT = P^T tile (stationary), rhs = V augmented with a ones
    column -> PSUM accumulates [q, 64 attn dims + denominator].
  - Normalize by per-partition reciprocal of the denominator, PE-transpose attn
    to [dim, tok], output projection against host-pre-transposed o_w columns.
Host: shards/pre-transposes/casts inputs, sums the 8 partial outputs.
"""
import sys

sys.path.insert(0, "/opt/trn_rl_repo")

import numpy as np
import ml_dtypes

import concourse.bass as bass
import concourse.mybir as mybir
import concourse.tile as tile
from concourse.bacc import Bacc
from concourse.bass_utils import run_bass_kernel_spmd
from concourse.masks import make_identity

B, S_FULL, H = 2, 2048, 2048
NH, HD, ROT = 32, 64, 16
THETA = 10000.0
NCORES = 8
HPC = NH // NCORES            # heads per core = 4
LDIM = HPC * HD               # local attn dims = 256
NEG = -1e30

bf16 = mybir.dt.bfloat16
f32 = mybir.dt.float32
nbf16 = ml_dtypes.bfloat16
Exp = mybir.ActivationFunctionType.Exp


# --------------------------------------------------------------------------
# Bass program (identical on every core; per-core tensors differ)
# --------------------------------------------------------------------------

def build_nc(S=S_FULL, debug=False):
    assert S % 512 == 0
    T = B * S
    TT = T // 128                 # token tiles total
    TPB = S // 128                # token tiles per batch
    NQB = S // 512                # 512-wide q blocks per batch
    HC = H // 128                 # h (contraction) chunks

    nc = Bacc()
    xT_d = nc.dram_tensor("xT", [H, T], bf16, kind="ExternalInput")
    wqkv_d = nc.dram_tensor("wqkvT", [H, 768], bf16, kind="ExternalInput")
    wo_d = nc.dram_tensor("woT", [LDIM, H], bf16, kind="ExternalInput")
    cs_d = nc.dram_tensor("csd", [128, TT * 2 * ROT], bf16, kind="ExternalInput")
    mask_d = nc.dram_tensor("maskd", [128, 128], bf16, kind="ExternalInput")
    out_d = nc.dram_tensor("out", [T, H], f32, kind="ExternalOutput")
    if debug:
        dbg_qkT = nc.dram_tensor("dbg_qkT", [128, 4, T], bf16, kind="ExternalOutput")
        dbg_V = nc.dram_tensor("dbg_V", [128, TT, HPC, 66], bf16, kind="ExternalOutput")
        dbg_aT = nc.dram_tensor("dbg_aT", [128, 2, T], bf16, kind="ExternalOutput")

    with tile.TileContext(nc) as tc:
        with tc.tile_pool(name="const", bufs=1) as cpool:
            wqkv_sb = cpool.tile([128, HC, 768], bf16)
            wqkv_r = wqkv_d.rearrange("(c p) d -> p c d", p=128)
            for q4 in range(4):   # 4 parallel DMAs across queues
                nc.sync.dma_start(
                    out=wqkv_sb[:, q4 * 4:(q4 + 1) * 4, :],
                    in_=wqkv_r[:, q4 * 4:(q4 + 1) * 4, :])
            wo_sb = cpool.tile([128, 2, H], bf16)
            nc.sync.dma_start(
                out=wo_sb, in_=wo_d.rearrange("(c p) d -> p c d", p=128))
            cs_sb = cpool.tile([128, TT, 2 * ROT], bf16)
            nc.sync.dma_start(
                out=cs_sb, in_=cs_d.rearrange("p (t d) -> p t d", d=2 * ROT))
            mask_sb = cpool.tile([128, 128], bf16)   # 0/1 keep-mask, applied post-exp
            nc.sync.dma_start(out=mask_sb, in_=mask_d[:, :])
            ident = cpool.tile([128, 128], bf16)
            make_identity(nc, ident)
            zeros_sb = cpool.tile([128, 512], bf16)
            nc.vector.memset(zeros_sb, 0.0)

            qkT_sb = cpool.tile([128, 4, T], bf16)     # dims x tok (4 dtiles)
            V_sb = cpool.tile([128, TT, HPC, 66], bf16)  # tok x head x (64+one)
            nc.vector.memset(V_sb[:, :, :, 64:65], 1.0)
            attnN_sb = cpool.tile([128, 2, T], bf16)   # normalized attn, q x dims

            # SBUF pools stay open for the whole kernel: closing them would
            # recycle SBUF bytes across phases and saddle the next phase's
            # first-touch instructions with WAR waits on every DMA queue
            # (walrus rejects >N sync waits per instruction). Only PSUM pools
            # are phase-scoped (PSUM deps never involve DMA queues).
            sb_pools = [
                tc.tile_pool(name="xt", bufs=4),
                tc.tile_pool(name="qknat", bufs=3),
                tc.tile_pool(name="ropetmp", bufs=4),
                tc.tile_pool(name="ppool", bufs=6),
                tc.tile_pool(name="anpool", bufs=4),
                tc.tile_pool(name="recpool", bufs=4),
                tc.tile_pool(name="obpool", bufs=4),
            ]
            xpool, qpool, rpool, ppool, anpool, recpool, obpool = \
                [p.__enter__() for p in sb_pools]

            # ---------------- Phase A: QKV projection + RoPE + transpose
            with tc.tile_pool(name="psA", bufs=2, space="PSUM") as psA, \
                 tc.tile_pool(name="tpA", bufs=3, space="PSUM") as tpA:
                xT_r = xT_d.rearrange("(c p) t -> p c t", p=128)
                for ti in range(TT):
                    xt = xpool.tile([128, HC, 128], bf16, tag="xt")
                    nc.sync.dma_start(
                        out=xt, in_=xT_r[:, :, ti * 128:(ti + 1) * 128])
                    qk_ps = psA.tile([128, 512], f32, tag="qk")
                    v_ps = psA.tile([128, 256], f32, tag="v")
                    for hc in range(HC):
                        nc.tensor.matmul(
                            qk_ps, xt[:, hc, :], wqkv_sb[:, hc, 0:512],
                            start=(hc == 0), stop=(hc == HC - 1))
                        nc.tensor.matmul(
                            v_ps, xt[:, hc, :], wqkv_sb[:, hc, 512:768],
                            start=(hc == 0), stop=(hc == HC - 1))
                    # V -> SBUF (bf16), ones col already set
                    nc.vector.tensor_copy(
                        V_sb[:, ti, :, 0:64],
                        v_ps.rearrange("p (h d) -> p h d", d=64))
                    # qk -> SBUF bf16 (DVE: ScalarE Copy measured ~8x slower)
                    qk = qpool.tile([128, 512], bf16, tag="qk")
                    nc.vector.tensor_copy(qk, qk_ps)
                    # partial RoPE on dims 0..15 of each of the 8 (q/k, head) blocks
                    rot = qk.rearrange("p (b d) -> p b d", d=64)[:, :, 0:ROT]
                    rot_lo = qk.rearrange("p (b d) -> p b d", d=64)[:, :, 0:8]
                    rot_hi = qk.rearrange("p (b d) -> p b d", d=64)[:, :, 8:16]
                    cos_bc = cs_sb[:, ti, None, 0:ROT].broadcast_to([128, 8, ROT])
                    sin_lo = cs_sb[:, ti, None, ROT:ROT + 8].broadcast_to([128, 8, 8])
                    sin_hi = cs_sb[:, ti, None, ROT + 8:ROT + 16].broadcast_to([128, 8, 8])
                    tmp = rpool.tile([128, 8, ROT], bf16, tag="t0")
                    t2l = rpool.tile([128, 8, 8], bf16, tag="t1")
                    t2h = rpool.tile([128, 8, 8], bf16, tag="t2")
                    nc.vector.tensor_mul(tmp, rot, cos_bc)
                    nc.vector.tensor_mul(t2l, rot_hi, sin_lo)
                    nc.vector.tensor_mul(t2h, rot_lo, sin_hi)
                    nc.vector.tensor_sub(rot_lo, tmp[:, :, 0:8], t2l)
                    nc.vector.tensor_add(rot_hi, tmp[:, :, 8:16], t2h)
                    # transpose the 4 dim-tiles into qkT
                    for dt in range(4):
                        tp = tpA.tile([128, 128], bf16, tag="tp")
                        nc.tensor.transpose(
                            tp, qk[:, dt * 128:(dt + 1) * 128], ident)
                        if dt % 2 == 0:
                            nc.vector.tensor_copy(
                                qkT_sb[:, dt, ti * 128:(ti + 1) * 128], tp)
                        else:
                            nc.scalar.copy(
                                qkT_sb[:, dt, ti * 128:(ti + 1) * 128], tp)

            def emit_outproj(ti_range, tp_pool, o_pool, use_act=False):
                for ti in ti_range:
                    tpt = tp_pool.tile([128, 2, 128], bf16, tag="tpt")
                    nc.tensor.transpose(
                        tpt[:, 0, :],
                        attnN_sb[:, 0, ti * 128:(ti + 1) * 128], ident)
                    nc.tensor.transpose(
                        tpt[:, 1, :],
                        attnN_sb[:, 1, ti * 128:(ti + 1) * 128], ident)
                    aT = anpool.tile([128, 2, 128], bf16, tag="an")
                    nc.vector.tensor_copy(aT, tpt)
                    for oc in range(4):
                        ops = o_pool.tile([128, 512], f32, tag="o")
                        nc.tensor.matmul(
                            ops, aT[:, 0, :],
                            wo_sb[:, 0, oc * 512:(oc + 1) * 512],
                            start=True, stop=False)
                        nc.tensor.matmul(
                            ops, aT[:, 1, :],
                            wo_sb[:, 1, oc * 512:(oc + 1) * 512],
                            start=False, stop=True)
                        ob = obpool.tile([128, 512], f32, tag="ob")
                        if use_act and oc % 2 == 0:
                            nc.scalar.copy(ob, ops)
                        else:
                            nc.vector.tensor_copy(ob, ops)
                        nc.sync.dma_start(
                            out=out_d[ti * 128:(ti + 1) * 128,
                                      oc * 512:(oc + 1) * 512],
                            in_=ob)

            # ---------------- Phase B: attention (per batch, per head pair).
            # spool=3 keeps the S matmuls two steps ahead of the exp stream
            # so ScalarE never stalls. PSUM: spool 3x2 + apool 2 = 8 banks.
            with tc.tile_pool(name="spool", bufs=3, space="PSUM") as spool, \
                 tc.tile_pool(name="apool", bufs=2, space="PSUM") as apool:
                for b in range(B):
                    for pr in range(2):          # head pairs (2pr, 2pr+1)
                        hA, hB = 2 * pr, 2 * pr + 1
                        for qb in range(NQB):
                            accA = apool.tile([128, 4, 128], f32, tag="acc")
                            accB = apool.tile([128, 4, 128], f32, tag="acc")
                            # open one accumulation group per bank: zero the
                            # whole bank so the interleaved per-q-tile PV
                            # accumulations below can all run with start=False
                            nc.tensor.matmul(
                                accA[:, :, :], ident, zeros_sb,
                                start=True, stop=False)
                            nc.tensor.matmul(
                                accB[:, :, :], ident, zeros_sb,
                                start=True, stop=False)
                            for ki in range(4 * qb + 4):
                                off = max(0, ki * 128 - qb * 512)
                                kcol = b * S + ki * 128
                                qcol = b * S + qb * 512
                                # both heads' score tiles in one 2-bank PSUM
                                # tile -> single exp / mask / sem per step.
                                # High priority: the S matmuls pace the exp
                                # stream on ScalarE; don't let interleaved
                                # out-proj matmuls delay them.
                                sAB = spool.tile([128, 2, 512], f32, tag="s")
                                with tc.high_priority(offset=40):
                                    nc.tensor.matmul(
                                        sAB[:, 0, off:512],
                                        qkT_sb[0:64, 2 + pr, kcol:kcol + 128],
                                        qkT_sb[0:64, pr, qcol + off:qcol + 512],
                                        start=True, stop=True,
                                        tile_position=(0, 0))
                                    nc.tensor.matmul(
                                        sAB[:, 1, off:512],
                                        qkT_sb[64:128, 2 + pr, kcol:kcol + 128],
                                        qkT_sb[64:128, pr, qcol + off:qcol + 512],
                                        start=True, stop=True,
                                        tile_position=(64, 0))
                                pAB = ppool.tile([128, 2, 512], bf16, tag="p")
                                nc.scalar.activation(
                                    out=pAB[:, :, off:512],
                                    in_=sAB[:, :, off:512], func=Exp)
                                if ki * 128 >= qb * 512:  # in-block diagonal:
                                    # causal-zero the P tile after exp (keeps
                                    # exp off the DVE dependency chain)
                                    mask2 = mask_sb[:, None, :].broadcast_to(
                                        [128, 2, 128])
                                    nc.vector.tensor_mul(
                                        pAB[:, :, off:off + 128],
                                        pAB[:, :, off:off + 128], mask2)
                                for j in range(4):
                                    qg = qb * 4 + j
                                    if qg < ki:
                                        continue
                                    last = (j == 3 and ki == 4 * qb + 3)
                                    nc.tensor.matmul(
                                        accA[:, j, 0:65],
                                        pAB[:, 0, j * 128:(j + 1) * 128],
                                        V_sb[:, b * TPB + ki, hA, 0:65],
                                        start=False, stop=last)
                                    nc.tensor.matmul(
                                        accB[:, j, 0:65],
                                        pAB[:, 1, j * 128:(j + 1) * 128],
                                        V_sb[:, b * TPB + ki, hB, 0:65],
                                        start=False, stop=last)
                            # normalize into attnN (q x dims layout; the
                            # transpose happens in the out-proj chunk)
                            for j in range(4):
                                recA = recpool.tile([128, 1], f32, tag="r")
                                recB = recpool.tile([128, 1], f32, tag="r")
                                nc.vector.reciprocal(recA, accA[:, j, 64:65])
                                nc.vector.reciprocal(recB, accB[:, j, 64:65])
                                col = b * S + (qb * 4 + j) * 128
                                nc.vector.tensor_scalar_mul(
                                    attnN_sb[:, pr, col:col + 64],
                                    accA[:, j, 0:64], recA)
                                nc.vector.tensor_scalar_mul(
                                    attnN_sb[:, pr, col + 64:col + 128],
                                    accB[:, j, 0:64], recB)

            # ---------------- Phase C: output projection, with the attention
            # PSUM pools released so copy/matmul chains pipeline wide.
            with tc.tile_pool(name="tpC2", bufs=2, space="PSUM") as tpC2, \
                 tc.tile_pool(name="opool2", bufs=4, space="PSUM") as opool2:
                emit_outproj(range(TT), tpC2, opool2, use_act=True)

            if debug:
                nc.sync.dma_start(out=dbg_qkT[:, :, :], in_=qkT_sb)
                nc.sync.dma_start(out=dbg_V[:, :, :, 0:65], in_=V_sb[:, :, :, 0:65])
                nc.sync.dma_start(out=dbg_aT[:, :, :], in_=attnN_sb)

            for p in reversed(sb_pools):
                p.__exit__(None, None, None)
    nc.finalize()
    return nc


# --------------------------------------------------------------------------
# Host-side prep
# --------------------------------------------------------------------------

def _host_prep(hidden_states, qkv_w, o_w, position_ids, S=S_FULL):
    """Returns (shared dict, per-core list of dicts) of numpy arrays."""
    T = B * S
    x = np.ascontiguousarray(hidden_states.reshape(T, H), dtype=np.float32)
    xT = np.ascontiguousarray(x.T).astype(nbf16)

    pos = np.asarray(position_ids).reshape(T).astype(np.float64)
    inv = THETA ** (-np.arange(0, ROT, 2, dtype=np.float64) / ROT)  # [8]
    f = pos[:, None] * inv[None, :]                                 # [T, 8]
    emb = np.concatenate([f, f], axis=1)                            # [T, 16]
    TT = T // 128
    # packed per-partition-linear layout [128, TT, 32]: cos | sin
    cs = np.empty((128, TT, 2 * ROT), np.float32)
    cs[:, :, 0:ROT] = np.cos(emb).reshape(TT, 128, ROT).transpose(1, 0, 2)
    cs[:, :, ROT:2 * ROT] = np.sin(emb).reshape(TT, 128, ROT).transpose(1, 0, 2)
    csd = np.ascontiguousarray(cs.reshape(128, TT * 2 * ROT)).astype(nbf16)

    # mask[p, j]: 1 when q offset j >= k offset p else 0 (applied to P post-exp)
    p_idx = np.arange(128)[:, None]
    j_idx = np.arange(128)[None, :]
    maskd = np.ascontiguousarray(
        np.where(j_idx >= p_idx, 1.0, 0.0)).astype(nbf16)

    shared = {"xT": xT, "csd": csd, "maskd": maskd}

    qkv = np.asarray(qkv_w, dtype=np.float32)
    ow = np.asarray(o_w, dtype=np.float32)
    scale = 1.0 / np.sqrt(HD)
    per_core = []
    for c in range(NCORES):
        cols = np.empty((768, H), np.float32)
        for t in range(4):                    # qk dim-tiles
            qk_sel = 0 if t < 2 else 1        # 0 = q, 1 = k
            for u in range(2):
                hl = 2 * (t % 2) + u
                hg = HPC * c + hl
                w = qkv[qk_sel * H + hg * HD: qk_sel * H + (hg + 1) * HD]
                if qk_sel == 0:
                    w = w * scale
                cols[t * 128 + u * 64: t * 128 + u * 64 + 64] = w
        for hl in range(HPC):                 # v dims
            hg = HPC * c + hl
            cols[512 + hl * 64: 512 + (hl + 1) * 64] = \
                qkv[2 * H + hg * HD: 2 * H + (hg + 1) * HD]
        wqkvT = np.ascontiguousarray(cols.T).astype(nbf16)
        woT = np.ascontiguousarray(
            ow[:, LDIM * c: LDIM * (c + 1)].T).astype(nbf16)
        per_core.append({"wqkvT": wqkvT, "woT": woT})
    return shared, per_core


_NC_CACHE = {}


def _get_nc(S=S_FULL):
    if S not in _NC_CACHE:
        _NC_CACHE[S] = build_nc(S)
    return _NC_CACHE[S]


def _run(hidden_states, qkv_w, o_w, position_ids, S=S_FULL, trace=False,
         trace_kwargs=None):
    shared, per_core = _host_prep(hidden_states, qkv_w, o_w, position_ids, S)
    in_maps = [{**shared, **per_core[c]} for c in range(NCORES)]
    nc = _get_nc(S)
    br = run_bass_kernel_spmd(
        nc, in_maps, list(range(NCORES)), trace=trace,
        **(trace_kwargs or {}))
    T = B * S
    out = np.zeros((T, H), np.float32)
    for r in br.results:
        out += r["out"]
    return out.reshape(B, S, H), br


def kernel(hidden_states, qkv_w, o_w, position_ids):
    out, _ = _run(hidden_states, qkv_w, o_w, position_ids)
    return out


# revision 51
# speedup vs baseline: 1.1313x; 1.1313x over previous
"""Trainium2 Bass kernel for GPT-NeoX-style attention block (nn_Attention_88141318848873).

Full inputs -> head-parallel tensor-parallel across 8 NeuronCores -> full output.

Per core c (local heads = global heads 4c..4c+3):
  - QKV projection in natural [tok, dim] layout (bf16 matmuls, fp32 PSUM accum),
    partial RoPE applied with free-dim strided DVE ops, then PE-transpose of q/k
    into [dim, tok] layout for the scores matmuls.
  - Scores computed transposed (S^T[k, q]) with two heads packed into the 128x128
    PE array via partition bases 0/64 (K=64 row tiling).
  - Softmax: no max subtraction (scores bounded ~|5| for this problem's scale),
    exp on ScalarE with causal block skipping, probabilities kept unnormalized.
  - PV flash-style: lhsT = P^T tile (stationary), rhs = V augmented with a ones
    column -> PSUM accumulates [q, 64 attn dims + denominator].
  - Normalize by per-partition reciprocal of the denominator, PE-transpose attn
    to [dim, tok], output projection against host-pre-transposed o_w columns.
Host: shards/pre-transposes/casts inputs, sums the 8 partial outputs.
"""
import sys

sys.path.insert(0, "/opt/trn_rl_repo")

import numpy as np
import ml_dtypes

import concourse.bass as bass
import concourse.mybir as mybir
import concourse.tile as tile
from concourse.bacc import Bacc
from concourse.bass_utils import run_bass_kernel_spmd
from concourse.masks import make_identity

B, S_FULL, H = 2, 2048, 2048
NH, HD, ROT = 32, 64, 16
THETA = 10000.0
NCORES = 8
HPC = NH // NCORES            # heads per core = 4
LDIM = HPC * HD               # local attn dims = 256
NEG = -1e30

bf16 = mybir.dt.bfloat16
f32 = mybir.dt.float32
nbf16 = ml_dtypes.bfloat16
Exp = mybir.ActivationFunctionType.Exp


# --------------------------------------------------------------------------
# Bass program (identical on every core; per-core tensors differ)
# --------------------------------------------------------------------------

def build_nc(S=S_FULL, debug=False):
    assert S % 512 == 0
    T = B * S
    TT = T // 128                 # token tiles total
    TPB = S // 128                # token tiles per batch
    NQB = S // 512                # 512-wide q blocks per batch
    HC = H // 128                 # h (contraction) chunks

    nc = Bacc()
    xT_d = nc.dram_tensor("xT", [H, T], bf16, kind="ExternalInput")
    wqkv_d = nc.dram_tensor("wqkvT", [H, 768], bf16, kind="ExternalInput")
    wo_d = nc.dram_tensor("woT", [LDIM, H], bf16, kind="ExternalInput")
    cs_d = nc.dram_tensor("csd", [128, TT * 2 * ROT], bf16, kind="ExternalInput")
    mask_d = nc.dram_tensor("maskd", [128, 128], bf16, kind="ExternalInput")
    out_d = nc.dram_tensor("out", [T, H], f32, kind="ExternalOutput")
    if debug:
        dbg_qkT = nc.dram_tensor("dbg_qkT", [128, 4, T], bf16, kind="ExternalOutput")
        dbg_V = nc.dram_tensor("dbg_V", [128, TT, HPC, 66], bf16, kind="ExternalOutput")
        dbg_aT = nc.dram_tensor("dbg_aT", [128, 2, T], bf16, kind="ExternalOutput")

    with tile.TileContext(nc) as tc:
        with tc.tile_pool(name="const", bufs=1) as cpool:
            wqkv_sb = cpool.tile([128, HC, 768], bf16)
            wqkv_r = wqkv_d.rearrange("(c p) d -> p c d", p=128)
            for q4 in range(4):   # 4 parallel DMAs across queues
                nc.sync.dma_start(
                    out=wqkv_sb[:, q4 * 4:(q4 + 1) * 4, :],
                    in_=wqkv_r[:, q4 * 4:(q4 + 1) * 4, :])
            wo_sb = cpool.tile([128, 2, H], bf16)
            nc.sync.dma_start(
                out=wo_sb, in_=wo_d.rearrange("(c p) d -> p c d", p=128))
            cs_sb = cpool.tile([128, TT, 2 * ROT], bf16)
            nc.sync.dma_start(
                out=cs_sb, in_=cs_d.rearrange("p (t d) -> p t d", d=2 * ROT))
            mask_sb = cpool.tile([128, 128], bf16)   # 0/1 keep-mask, applied post-exp
            nc.sync.dma_start(out=mask_sb, in_=mask_d[:, :])
            ident = cpool.tile([128, 128], bf16)
            make_identity(nc, ident)
            zeros_sb = cpool.tile([128, 512], bf16)
            nc.vector.memset(zeros_sb, 0.0)

            qkT_sb = cpool.tile([128, 4, T], bf16)     # dims x tok (4 dtiles)
            V_sb = cpool.tile([128, TT, HPC, 66], bf16)  # tok x head x (64+one)
            nc.vector.memset(V_sb[:, :, :, 64:65], 1.0)
            attnN_sb = cpool.tile([128, 2, T], bf16)   # normalized attn, q x dims

            # SBUF pools stay open for the whole kernel: closing them would
            # recycle SBUF bytes across phases and saddle the next phase's
            # first-touch instructions with WAR waits on every DMA queue
            # (walrus rejects >N sync waits per instruction). Only PSUM pools
            # are phase-scoped (PSUM deps never involve DMA queues).
            sb_pools = [
                tc.tile_pool(name="xt", bufs=4),
                tc.tile_pool(name="qknat", bufs=3),
                tc.tile_pool(name="ropetmp", bufs=4),
                tc.tile_pool(name="ppool", bufs=6),
                tc.tile_pool(name="anpool", bufs=4),
                tc.tile_pool(name="recpool", bufs=4),
                tc.tile_pool(name="obpool", bufs=4),
            ]
            xpool, qpool, rpool, ppool, anpool, recpool, obpool = \
                [p.__enter__() for p in sb_pools]

            # ---------------- Phase A: QKV projection + RoPE + transpose
            with tc.tile_pool(name="psA", bufs=2, space="PSUM") as psA, \
                 tc.tile_pool(name="tpA", bufs=3, space="PSUM") as tpA:
                xT_r = xT_d.rearrange("(c p) t -> p c t", p=128)
                for ti in range(TT):
                    xt = xpool.tile([128, HC, 128], bf16, tag="xt")
                    nc.sync.dma_start(
                        out=xt, in_=xT_r[:, :, ti * 128:(ti + 1) * 128])
                    qk_ps = psA.tile([128, 512], f32, tag="qk")
                    v_ps = psA.tile([128, 256], f32, tag="v")
                    for hc in range(HC):
                        nc.tensor.matmul(
                            qk_ps, xt[:, hc, :], wqkv_sb[:, hc, 0:512],
                            start=(hc == 0), stop=(hc == HC - 1))
                        nc.tensor.matmul(
                            v_ps, xt[:, hc, :], wqkv_sb[:, hc, 512:768],
                            start=(hc == 0), stop=(hc == HC - 1))
                    # V -> SBUF (bf16), ones col already set
                    nc.vector.tensor_copy(
                        V_sb[:, ti, :, 0:64],
                        v_ps.rearrange("p (h d) -> p h d", d=64))
                    # qk -> SBUF bf16 (DVE: ScalarE Copy measured ~8x slower)
                    qk = qpool.tile([128, 512], bf16, tag="qk")
                    nc.vector.tensor_copy(qk, qk_ps)
                    # partial RoPE on dims 0..15 of each of the 8 (q/k, head) blocks
                    rot = qk.rearrange("p (b d) -> p b d", d=64)[:, :, 0:ROT]
                    rot_lo = qk.rearrange("p (b d) -> p b d", d=64)[:, :, 0:8]
                    rot_hi = qk.rearrange("p (b d) -> p b d", d=64)[:, :, 8:16]
                    cos_bc = cs_sb[:, ti, None, 0:ROT].broadcast_to([128, 8, ROT])
                    sin_lo = cs_sb[:, ti, None, ROT:ROT + 8].broadcast_to([128, 8, 8])
                    sin_hi = cs_sb[:, ti, None, ROT + 8:ROT + 16].broadcast_to([128, 8, 8])
                    tmp = rpool.tile([128, 8, ROT], bf16, tag="t0")
                    t2l = rpool.tile([128, 8, 8], bf16, tag="t1")
                    t2h = rpool.tile([128, 8, 8], bf16, tag="t2")
                    nc.vector.tensor_mul(tmp, rot, cos_bc)
                    nc.vector.tensor_mul(t2l, rot_hi, sin_lo)
                    nc.vector.tensor_mul(t2h, rot_lo, sin_hi)
                    nc.vector.tensor_sub(rot_lo, tmp[:, :, 0:8], t2l)
                    nc.vector.tensor_add(rot_hi, tmp[:, :, 8:16], t2h)
                    # transpose the 4 dim-tiles into qkT
                    for dt in range(4):
                        tp = tpA.tile([128, 128], bf16, tag="tp")
                        nc.tensor.transpose(
                            tp, qk[:, dt * 128:(dt + 1) * 128], ident)
                        if dt % 2 == 0:
                            nc.vector.tensor_copy(
                                qkT_sb[:, dt, ti * 128:(ti + 1) * 128], tp)
                        else:
                            nc.scalar.copy(
                                qkT_sb[:, dt, ti * 128:(ti + 1) * 128], tp)

            def emit_outproj(ti_range, tp_pool, o_pool, use_act=False):
                for ti in ti_range:
                    tpt = tp_pool.tile([128, 2, 128], bf16, tag="tpt")
                    nc.tensor.transpose(
                        tpt[:, 0, :],
                        attnN_sb[:, 0, ti * 128:(ti + 1) * 128], ident)
                    nc.tensor.transpose(
                        tpt[:, 1, :],
                        attnN_sb[:, 1, ti * 128:(ti + 1) * 128], ident)
                    aT = anpool.tile([128, 2, 128], bf16, tag="an")
                    nc.vector.tensor_copy(aT, tpt)
                    for oc in range(4):
                        ops = o_pool.tile([128, 512], f32, tag="o")
                        nc.tensor.matmul(
                            ops, aT[:, 0, :],
                            wo_sb[:, 0, oc * 512:(oc + 1) * 512],
                            start=True, stop=False)
                        nc.tensor.matmul(
                            ops, aT[:, 1, :],
                            wo_sb[:, 1, oc * 512:(oc + 1) * 512],
                            start=False, stop=True)
                        ob = obpool.tile([128, 512], f32, tag="ob")
                        if use_act and oc % 2 == 0:
                            nc.scalar.copy(ob, ops)
                        else:
                            nc.vector.tensor_copy(ob, ops)
                        nc.sync.dma_start(
                            out=out_d[ti * 128:(ti + 1) * 128,
                                      oc * 512:(oc + 1) * 512],
                            in_=ob)

            # ---------------- Phase B: attention (per batch, per head pair),
            # with early out-proj chunks interleaved (they fill PE slack in
            # the exp-paced pipeline); late chunks go post-B where the freed
            # PSUM lets them pipeline wide.
            # PSUM budget: spool 2x2 + apool 2 + tpC 1 + opool 1 = 8 banks.
            with tc.tile_pool(name="spool", bufs=2, space="PSUM") as spool, \
                 tc.tile_pool(name="apool", bufs=2, space="PSUM") as apool, \
                 tc.tile_pool(name="tpC", bufs=1, space="PSUM") as tpC, \
                 tc.tile_pool(name="opool", bufs=1, space="PSUM") as opool:
                for b in range(B):
                    for pr in range(2):          # head pairs (2pr, 2pr+1)
                        hA, hB = 2 * pr, 2 * pr + 1
                        for qb in range(NQB):
                            accA = apool.tile([128, 4, 128], f32, tag="acc")
                            accB = apool.tile([128, 4, 128], f32, tag="acc")
                            # open one accumulation group per bank: zero the
                            # whole bank so the interleaved per-q-tile PV
                            # accumulations below can all run with start=False
                            nc.tensor.matmul(
                                accA[:, :, :], ident, zeros_sb,
                                start=True, stop=False)
                            nc.tensor.matmul(
                                accB[:, :, :], ident, zeros_sb,
                                start=True, stop=False)
                            for ki in range(4 * qb + 4):
                                off = max(0, ki * 128 - qb * 512)
                                kcol = b * S + ki * 128
                                qcol = b * S + qb * 512
                                # both heads' score tiles in one 2-bank PSUM
                                # tile -> single exp / mask / sem per step.
                                # High priority: the S matmuls pace the exp
                                # stream on ScalarE; don't let interleaved
                                # out-proj matmuls delay them.
                                sAB = spool.tile([128, 2, 512], f32, tag="s")
                                with tc.high_priority(offset=40):
                                    nc.tensor.matmul(
                                        sAB[:, 0, off:512],
                                        qkT_sb[0:64, 2 + pr, kcol:kcol + 128],
                                        qkT_sb[0:64, pr, qcol + off:qcol + 512],
                                        start=True, stop=True,
                                        tile_position=(0, 0))
                                    nc.tensor.matmul(
                                        sAB[:, 1, off:512],
                                        qkT_sb[64:128, 2 + pr, kcol:kcol + 128],
                                        qkT_sb[64:128, pr, qcol + off:qcol + 512],
                                        start=True, stop=True,
                                        tile_position=(64, 0))
                                pAB = ppool.tile([128, 2, 512], bf16, tag="p")
                                nc.scalar.activation(
                                    out=pAB[:, :, off:512],
                                    in_=sAB[:, :, off:512], func=Exp)
                                if ki * 128 >= qb * 512:  # in-block diagonal:
                                    # causal-zero the P tile after exp (keeps
                                    # exp off the DVE dependency chain)
                                    mask2 = mask_sb[:, None, :].broadcast_to(
                                        [128, 2, 128])
                                    nc.vector.tensor_mul(
                                        pAB[:, :, off:off + 128],
                                        pAB[:, :, off:off + 128], mask2)
                                for j in range(4):
                                    qg = qb * 4 + j
                                    if qg < ki:
                                        continue
                                    last = (j == 3 and ki == 4 * qb + 3)
                                    nc.tensor.matmul(
                                        accA[:, j, 0:65],
                                        pAB[:, 0, j * 128:(j + 1) * 128],
                                        V_sb[:, b * TPB + ki, hA, 0:65],
                                        start=False, stop=last)
                                    nc.tensor.matmul(
                                        accB[:, j, 0:65],
                                        pAB[:, 1, j * 128:(j + 1) * 128],
                                        V_sb[:, b * TPB + ki, hB, 0:65],
                                        start=False, stop=last)
                            # normalize into attnN (q x dims layout; the
                            # transpose happens in the out-proj chunk)
                            for j in range(4):
                                recA = recpool.tile([128, 1], f32, tag="r")
                                recB = recpool.tile([128, 1], f32, tag="r")
                                nc.vector.reciprocal(recA, accA[:, j, 64:65])
                                nc.vector.reciprocal(recB, accB[:, j, 64:65])
                                col = b * S + (qb * 4 + j) * 128
                                nc.vector.tensor_scalar_mul(
                                    attnN_sb[:, pr, col:col + 64],
                                    accA[:, j, 0:64], recA)
                                nc.vector.tensor_scalar_mul(
                                    attnN_sb[:, pr, col + 64:col + 128],
                                    accB[:, j, 0:64], recB)

                            if pr != 1:
                                continue
                            if b == B - 1 and qb >= NQB - 2:
                                continue   # late chunks: post-B, wide pools
                            emit_outproj(range(b * TPB + qb * 4,
                                               b * TPB + qb * 4 + 4),
                                         tpC, opool)

            # late out-proj chunks with the attention PSUM pools released
            with tc.tile_pool(name="tpC2", bufs=2, space="PSUM") as tpC2, \
                 tc.tile_pool(name="opool2", bufs=4, space="PSUM") as opool2:
                emit_outproj(range(TT - 8, TT), tpC2, opool2, use_act=True)

            if debug:
                nc.sync.dma_start(out=dbg_qkT[:, :, :], in_=qkT_sb)
                nc.sync.dma_start(out=dbg_V[:, :, :, 0:65], in_=V_sb[:, :, :, 0:65])
                nc.sync.dma_start(out=dbg_aT[:, :, :], in_=attnN_sb)

            for p in reversed(sb_pools):
                p.__exit__(None, None, None)
    nc.finalize()
    return nc


# --------------------------------------------------------------------------
# Host-side prep
# --------------------------------------------------------------------------

def _host_prep(hidden_states, qkv_w, o_w, position_ids, S=S_FULL):
    """Returns (shared dict, per-core list of dicts) of numpy arrays."""
    T = B * S
    x = np.ascontiguousarray(hidden_states.reshape(T, H), dtype=np.float32)
    xT = np.ascontiguousarray(x.T).astype(nbf16)

    pos = np.asarray(position_ids).reshape(T).astype(np.float64)
    inv = THETA ** (-np.arange(0, ROT, 2, dtype=np.float64) / ROT)  # [8]
    f = pos[:, None] * inv[None, :]                                 # [T, 8]
    emb = np.concatenate([f, f], axis=1)                            # [T, 16]
    TT = T // 128
    # packed per-partition-linear layout [128, TT, 32]: cos | sin
    cs = np.empty((128, TT, 2 * ROT), np.float32)
    cs[:, :, 0:ROT] = np.cos(emb).reshape(TT, 128, ROT).transpose(1, 0, 2)
    cs[:, :, ROT:2 * ROT] = np.sin(emb).reshape(TT, 128, ROT).transpose(1, 0, 2)
    csd = np.ascontiguousarray(cs.reshape(128, TT * 2 * ROT)).astype(nbf16)

    # mask[p, j]: 1 when q offset j >= k offset p else 0 (applied to P post-exp)
    p_idx = np.arange(128)[:, None]
    j_idx = np.arange(128)[None, :]
    maskd = np.ascontiguousarray(
        np.where(j_idx >= p_idx, 1.0, 0.0)).astype(nbf16)

    shared = {"xT": xT, "csd": csd, "maskd": maskd}

    qkv = np.asarray(qkv_w, dtype=np.float32)
    ow = np.asarray(o_w, dtype=np.float32)
    scale = 1.0 / np.sqrt(HD)
    per_core = []
    for c in range(NCORES):
        cols = np.empty((768, H), np.float32)
        for t in range(4):                    # qk dim-tiles
            qk_sel = 0 if t < 2 else 1        # 0 = q, 1 = k
            for u in range(2):
                hl = 2 * (t % 2) + u
                hg = HPC * c + hl
                w = qkv[qk_sel * H + hg * HD: qk_sel * H + (hg + 1) * HD]
                if qk_sel == 0:
                    w = w * scale
                cols[t * 128 + u * 64: t * 128 + u * 64 + 64] = w
        for hl in range(HPC):                 # v dims
            hg = HPC * c + hl
            cols[512 + hl * 64: 512 + (hl + 1) * 64] = \
                qkv[2 * H + hg * HD: 2 * H + (hg + 1) * HD]
        wqkvT = np.ascontiguousarray(cols.T).astype(nbf16)
        woT = np.ascontiguousarray(
            ow[:, LDIM * c: LDIM * (c + 1)].T).astype(nbf16)
        per_core.append({"wqkvT": wqkvT, "woT": woT})
    return shared, per_core


_NC_CACHE = {}


def _get_nc(S=S_FULL):
    if S not in _NC_CACHE:
        _NC_CACHE[S] = build_nc(S)
    return _NC_CACHE[S]


def _run(hidden_states, qkv_w, o_w, position_ids, S=S_FULL, trace=False,
         trace_kwargs=None):
    shared, per_core = _host_prep(hidden_states, qkv_w, o_w, position_ids, S)
    in_maps = [{**shared, **per_core[c]} for c in range(NCORES)]
    nc = _get_nc(S)
    br = run_bass_kernel_spmd(
        nc, in_maps, list(range(NCORES)), trace=trace,
        **(trace_kwargs or {}))
    T = B * S
    out = np.zeros((T, H), np.float32)
    for r in br.results:
        out += r["out"]
    return out.reshape(B, S, H), br


def kernel(hidden_states, qkv_w, o_w, position_ids):
    out, _ = _run(hidden_states, qkv_w, o_w, position_ids)
    return out


# revision 54
# speedup vs baseline: 1.1614x; 1.0266x over previous
"""Trainium2 Bass kernel for GPT-NeoX-style attention block (nn_Attention_88141318848873).

Full inputs -> head-parallel tensor-parallel across 8 NeuronCores -> full output.

Per core c (local heads = global heads 4c..4c+3):
  - QKV projection in natural [tok, dim] layout (bf16 matmuls, fp32 PSUM accum),
    partial RoPE applied with free-dim strided DVE ops, then PE-transpose of q/k
    into [dim, tok] layout for the scores matmuls.
  - Scores computed transposed (S^T[k, q]) with two heads packed into the 128x128
    PE array via partition bases 0/64 (K=64 row tiling).
  - Softmax: no max subtraction (scores bounded ~|5| for this problem's scale),
    exp on ScalarE with causal block skipping, probabilities kept unnormalized.
  - PV flash-style: lhsT = P^T tile (stationary), rhs = V augmented with a ones
    column -> PSUM accumulates [q, 64 attn dims + denominator].
  - Normalize by per-partition reciprocal of the denominator, PE-transpose attn
    to [dim, tok], output projection against host-pre-transposed o_w columns.
Host: shards/pre-transposes/casts inputs, sums the 8 partial outputs.
"""
import sys

sys.path.insert(0, "/opt/trn_rl_repo")

import numpy as np
import ml_dtypes

import concourse.bass as bass
import concourse.mybir as mybir
import concourse.tile as tile
from concourse.bacc import Bacc
from concourse.bass_utils import run_bass_kernel_spmd
from concourse.masks import make_identity

B, S_FULL, H = 2, 2048, 2048
NH, HD, ROT = 32, 64, 16
THETA = 10000.0
NCORES = 8
HPC = NH // NCORES            # heads per core = 4
LDIM = HPC * HD               # local attn dims = 256
NEG = -1e30

bf16 = mybir.dt.bfloat16
f32 = mybir.dt.float32
nbf16 = ml_dtypes.bfloat16
Exp = mybir.ActivationFunctionType.Exp


# --------------------------------------------------------------------------
# Bass program (identical on every core; per-core tensors differ)
# --------------------------------------------------------------------------

def build_nc(S=S_FULL, debug=False):
    assert S % 512 == 0
    T = B * S
    TT = T // 128                 # token tiles total
    TPB = S // 128                # token tiles per batch
    NQB = S // 512                # 512-wide q blocks per batch
    HC = H // 128                 # h (contraction) chunks

    nc = Bacc()
    xT_d = nc.dram_tensor("xT", [H, T], bf16, kind="ExternalInput")
    wqkv_d = nc.dram_tensor("wqkvT", [H, 768], bf16, kind="ExternalInput")
    wo_d = nc.dram_tensor("woT", [LDIM, H], bf16, kind="ExternalInput")
    cs_d = nc.dram_tensor("csd", [128, TT * 2 * ROT], bf16, kind="ExternalInput")
    mask_d = nc.dram_tensor("maskd", [128, 128], bf16, kind="ExternalInput")
    out_d = nc.dram_tensor("out", [T, H], bf16, kind="ExternalOutput")
    if debug:
        dbg_qkT = nc.dram_tensor("dbg_qkT", [128, 4, T], bf16, kind="ExternalOutput")
        dbg_V = nc.dram_tensor("dbg_V", [128, TT, HPC, 66], bf16, kind="ExternalOutput")
        dbg_aT = nc.dram_tensor("dbg_aT", [128, 2, T], bf16, kind="ExternalOutput")

    with tile.TileContext(nc) as tc:
        with tc.tile_pool(name="const", bufs=1) as cpool:
            wqkv_sb = cpool.tile([128, HC, 768], bf16)
            wqkv_r = wqkv_d.rearrange("(c p) d -> p c d", p=128)
            for q4 in range(4):   # 4 parallel DMAs across queues
                nc.sync.dma_start(
                    out=wqkv_sb[:, q4 * 4:(q4 + 1) * 4, :],
                    in_=wqkv_r[:, q4 * 4:(q4 + 1) * 4, :])
            wo_sb = cpool.tile([128, 2, H], bf16)
            nc.sync.dma_start(
                out=wo_sb, in_=wo_d.rearrange("(c p) d -> p c d", p=128))
            cs_sb = cpool.tile([128, TT, 2 * ROT], bf16)
            nc.sync.dma_start(
                out=cs_sb, in_=cs_d.rearrange("p (t d) -> p t d", d=2 * ROT))
            mask_sb = cpool.tile([128, 128], bf16)   # 0/1 keep-mask, applied post-exp
            nc.sync.dma_start(out=mask_sb, in_=mask_d[:, :])
            ident = cpool.tile([128, 128], bf16)
            make_identity(nc, ident)
            zeros_sb = cpool.tile([128, 512], bf16)
            nc.vector.memset(zeros_sb, 0.0)

            qkT_sb = cpool.tile([128, 4, T], bf16)     # dims x tok (4 dtiles)
            V_sb = cpool.tile([128, TT, HPC, 66], bf16)  # tok x head x (64+one)
            nc.vector.memset(V_sb[:, :, :, 64:65], 1.0)
            attnN_sb = cpool.tile([128, 2, T], bf16)   # normalized attn, q x dims

            # SBUF pools stay open for the whole kernel: closing them would
            # recycle SBUF bytes across phases and saddle the next phase's
            # first-touch instructions with WAR waits on every DMA queue
            # (walrus rejects >N sync waits per instruction). Only PSUM pools
            # are phase-scoped (PSUM deps never involve DMA queues).
            sb_pools = [
                tc.tile_pool(name="xt", bufs=4),
                tc.tile_pool(name="qknat", bufs=3),
                tc.tile_pool(name="ropetmp", bufs=4),
                tc.tile_pool(name="ppool", bufs=6),
                tc.tile_pool(name="anpool", bufs=4),
                tc.tile_pool(name="recpool", bufs=4),
                tc.tile_pool(name="obpool", bufs=6),
            ]
            xpool, qpool, rpool, ppool, anpool, recpool, obpool = \
                [p.__enter__() for p in sb_pools]

            # ---------------- Phase A: QKV projection + RoPE + transpose
            with tc.tile_pool(name="psA", bufs=2, space="PSUM") as psA, \
                 tc.tile_pool(name="tpA", bufs=3, space="PSUM") as tpA:
                xT_r = xT_d.rearrange("(c p) t -> p c t", p=128)
                for ti in range(TT):
                    xt = xpool.tile([128, HC, 128], bf16, tag="xt")
                    nc.sync.dma_start(
                        out=xt, in_=xT_r[:, :, ti * 128:(ti + 1) * 128])
                    qk_ps = psA.tile([128, 512], f32, tag="qk")
                    v_ps = psA.tile([128, 256], f32, tag="v")
                    for hc in range(HC):
                        nc.tensor.matmul(
                            qk_ps, xt[:, hc, :], wqkv_sb[:, hc, 0:512],
                            start=(hc == 0), stop=(hc == HC - 1))
                        nc.tensor.matmul(
                            v_ps, xt[:, hc, :], wqkv_sb[:, hc, 512:768],
                            start=(hc == 0), stop=(hc == HC - 1))
                    # V -> SBUF (bf16), ones col already set
                    nc.vector.tensor_copy(
                        V_sb[:, ti, :, 0:64],
                        v_ps.rearrange("p (h d) -> p h d", d=64))
                    # qk -> SBUF bf16 (DVE: ScalarE Copy measured ~8x slower)
                    qk = qpool.tile([128, 512], bf16, tag="qk")
                    nc.vector.tensor_copy(qk, qk_ps)
                    # partial RoPE on dims 0..15 of each of the 8 (q/k, head) blocks
                    rot = qk.rearrange("p (b d) -> p b d", d=64)[:, :, 0:ROT]
                    rot_lo = qk.rearrange("p (b d) -> p b d", d=64)[:, :, 0:8]
                    rot_hi = qk.rearrange("p (b d) -> p b d", d=64)[:, :, 8:16]
                    cos_bc = cs_sb[:, ti, None, 0:ROT].broadcast_to([128, 8, ROT])
                    sin_lo = cs_sb[:, ti, None, ROT:ROT + 8].broadcast_to([128, 8, 8])
                    sin_hi = cs_sb[:, ti, None, ROT + 8:ROT + 16].broadcast_to([128, 8, 8])
                    tmp = rpool.tile([128, 8, ROT], bf16, tag="t0")
                    t2l = rpool.tile([128, 8, 8], bf16, tag="t1")
                    t2h = rpool.tile([128, 8, 8], bf16, tag="t2")
                    nc.vector.tensor_mul(tmp, rot, cos_bc)
                    nc.vector.tensor_mul(t2l, rot_hi, sin_lo)
                    nc.vector.tensor_mul(t2h, rot_lo, sin_hi)
                    nc.vector.tensor_sub(rot_lo, tmp[:, :, 0:8], t2l)
                    nc.vector.tensor_add(rot_hi, tmp[:, :, 8:16], t2h)
                    # transpose the 4 dim-tiles into qkT
                    for dt in range(4):
                        tp = tpA.tile([128, 128], bf16, tag="tp")
                        nc.tensor.transpose(
                            tp, qk[:, dt * 128:(dt + 1) * 128], ident)
                        if dt % 2 == 0:
                            nc.vector.tensor_copy(
                                qkT_sb[:, dt, ti * 128:(ti + 1) * 128], tp)
                        else:
                            nc.scalar.copy(
                                qkT_sb[:, dt, ti * 128:(ti + 1) * 128], tp)

            def emit_outproj(ti_range, tp_pool, o_pool, use_act=False):
                for ti in ti_range:
                    tpt = tp_pool.tile([128, 2, 128], bf16, tag="tpt")
                    nc.tensor.transpose(
                        tpt[:, 0, :],
                        attnN_sb[:, 0, ti * 128:(ti + 1) * 128], ident)
                    nc.tensor.transpose(
                        tpt[:, 1, :],
                        attnN_sb[:, 1, ti * 128:(ti + 1) * 128], ident)
                    aT = anpool.tile([128, 2, 128], bf16, tag="an")
                    nc.vector.tensor_copy(aT, tpt)
                    for oc in range(4):
                        ops = o_pool.tile([128, 512], f32, tag="o")
                        nc.tensor.matmul(
                            ops, aT[:, 0, :],
                            wo_sb[:, 0, oc * 512:(oc + 1) * 512],
                            start=True, stop=False)
                        nc.tensor.matmul(
                            ops, aT[:, 1, :],
                            wo_sb[:, 1, oc * 512:(oc + 1) * 512],
                            start=False, stop=True)
                        ob = obpool.tile([128, 512], bf16, tag="ob")
                        if use_act and oc % 2 == 0:
                            nc.scalar.copy(ob, ops)
                        else:
                            nc.vector.tensor_copy(ob, ops)
                        nc.sync.dma_start(
                            out=out_d[ti * 128:(ti + 1) * 128,
                                      oc * 512:(oc + 1) * 512],
                            in_=ob)

            # ---------------- Phase B: attention (per batch, per head pair),
            # with early out-proj chunks interleaved (they fill PE slack in
            # the exp-paced pipeline); late chunks go post-B where the freed
            # PSUM lets them pipeline wide.
            # PSUM budget: spool 2x2 + apool 2 + tpC 1 + opool 1 = 8 banks.
            with tc.tile_pool(name="spool", bufs=2, space="PSUM") as spool, \
                 tc.tile_pool(name="apool", bufs=2, space="PSUM") as apool, \
                 tc.tile_pool(name="tpC", bufs=1, space="PSUM") as tpC, \
                 tc.tile_pool(name="opool", bufs=1, space="PSUM") as opool:
                for b in range(B):
                    for pr in range(2):          # head pairs (2pr, 2pr+1)
                        hA, hB = 2 * pr, 2 * pr + 1
                        for qb in range(NQB):
                            accA = apool.tile([128, 4, 128], f32, tag="acc")
                            accB = apool.tile([128, 4, 128], f32, tag="acc")
                            # open one accumulation group per bank: zero the
                            # whole bank so the interleaved per-q-tile PV
                            # accumulations below can all run with start=False
                            nc.tensor.matmul(
                                accA[:, :, :], ident, zeros_sb,
                                start=True, stop=False)
                            nc.tensor.matmul(
                                accB[:, :, :], ident, zeros_sb,
                                start=True, stop=False)
                            for ki in range(4 * qb + 4):
                                off = max(0, ki * 128 - qb * 512)
                                kcol = b * S + ki * 128
                                qcol = b * S + qb * 512
                                # both heads' score tiles in one 2-bank PSUM
                                # tile -> single exp / mask / sem per step.
                                # High priority: the S matmuls pace the exp
                                # stream on ScalarE; don't let interleaved
                                # out-proj matmuls delay them.
                                sAB = spool.tile([128, 2, 512], f32, tag="s")
                                with tc.high_priority(offset=40):
                                    nc.tensor.matmul(
                                        sAB[:, 0, off:512],
                                        qkT_sb[0:64, 2 + pr, kcol:kcol + 128],
                                        qkT_sb[0:64, pr, qcol + off:qcol + 512],
                                        start=True, stop=True,
                                        tile_position=(0, 0))
                                    nc.tensor.matmul(
                                        sAB[:, 1, off:512],
                                        qkT_sb[64:128, 2 + pr, kcol:kcol + 128],
                                        qkT_sb[64:128, pr, qcol + off:qcol + 512],
                                        start=True, stop=True,
                                        tile_position=(64, 0))
                                pAB = ppool.tile([128, 2, 512], bf16, tag="p")
                                nc.scalar.activation(
                                    out=pAB[:, :, off:512],
                                    in_=sAB[:, :, off:512], func=Exp)
                                if ki * 128 >= qb * 512:  # in-block diagonal:
                                    # causal-zero the P tile after exp (keeps
                                    # exp off the DVE dependency chain)
                                    mask2 = mask_sb[:, None, :].broadcast_to(
                                        [128, 2, 128])
                                    nc.vector.tensor_mul(
                                        pAB[:, :, off:off + 128],
                                        pAB[:, :, off:off + 128], mask2)
                                for j in range(4):
                                    qg = qb * 4 + j
                                    if qg < ki:
                                        continue
                                    last = (j == 3 and ki == 4 * qb + 3)
                                    nc.tensor.matmul(
                                        accA[:, j, 0:65],
                                        pAB[:, 0, j * 128:(j + 1) * 128],
                                        V_sb[:, b * TPB + ki, hA, 0:65],
                                        start=False, stop=last)
                                    nc.tensor.matmul(
                                        accB[:, j, 0:65],
                                        pAB[:, 1, j * 128:(j + 1) * 128],
                                        V_sb[:, b * TPB + ki, hB, 0:65],
                                        start=False, stop=last)
                            # normalize into attnN (q x dims layout; the
                            # transpose happens in the out-proj chunk)
                            for j in range(4):
                                recA = recpool.tile([128, 1], f32, tag="r")
                                recB = recpool.tile([128, 1], f32, tag="r")
                                nc.vector.reciprocal(recA, accA[:, j, 64:65])
                                nc.vector.reciprocal(recB, accB[:, j, 64:65])
                                col = b * S + (qb * 4 + j) * 128
                                nc.vector.tensor_scalar_mul(
                                    attnN_sb[:, pr, col:col + 64],
                                    accA[:, j, 0:64], recA)
                                nc.vector.tensor_scalar_mul(
                                    attnN_sb[:, pr, col + 64:col + 128],
                                    accB[:, j, 0:64], recB)

                            if pr != 1:
                                continue
                            if b == B - 1 and qb >= NQB - 2:
                                continue   # late chunks: post-B, wide pools
                            emit_outproj(range(b * TPB + qb * 4,
                                               b * TPB + qb * 4 + 4),
                                         tpC, opool)

            # late out-proj chunks with the attention PSUM pools released
            with tc.tile_pool(name="tpC2", bufs=2, space="PSUM") as tpC2, \
                 tc.tile_pool(name="opool2", bufs=4, space="PSUM") as opool2:
                emit_outproj(range(TT - 8, TT), tpC2, opool2, use_act=True)

            if debug:
                nc.sync.dma_start(out=dbg_qkT[:, :, :], in_=qkT_sb)
                nc.sync.dma_start(out=dbg_V[:, :, :, 0:65], in_=V_sb[:, :, :, 0:65])
                nc.sync.dma_start(out=dbg_aT[:, :, :], in_=attnN_sb)

            for p in reversed(sb_pools):
                p.__exit__(None, None, None)
    nc.finalize()
    return nc


# --------------------------------------------------------------------------
# Host-side prep
# --------------------------------------------------------------------------

def _host_prep(hidden_states, qkv_w, o_w, position_ids, S=S_FULL):
    """Returns (shared dict, per-core list of dicts) of numpy arrays."""
    T = B * S
    x = np.ascontiguousarray(hidden_states.reshape(T, H), dtype=np.float32)
    xT = np.ascontiguousarray(x.T).astype(nbf16)

    pos = np.asarray(position_ids).reshape(T).astype(np.float64)
    inv = THETA ** (-np.arange(0, ROT, 2, dtype=np.float64) / ROT)  # [8]
    f = pos[:, None] * inv[None, :]                                 # [T, 8]
    emb = np.concatenate([f, f], axis=1)                            # [T, 16]
    TT = T // 128
    # packed per-partition-linear layout [128, TT, 32]: cos | sin
    cs = np.empty((128, TT, 2 * ROT), np.float32)
    cs[:, :, 0:ROT] = np.cos(emb).reshape(TT, 128, ROT).transpose(1, 0, 2)
    cs[:, :, ROT:2 * ROT] = np.sin(emb).reshape(TT, 128, ROT).transpose(1, 0, 2)
    csd = np.ascontiguousarray(cs.reshape(128, TT * 2 * ROT)).astype(nbf16)

    # mask[p, j]: 1 when q offset j >= k offset p else 0 (applied to P post-exp)
    p_idx = np.arange(128)[:, None]
    j_idx = np.arange(128)[None, :]
    maskd = np.ascontiguousarray(
        np.where(j_idx >= p_idx, 1.0, 0.0)).astype(nbf16)

    shared = {"xT": xT, "csd": csd, "maskd": maskd}

    qkv = np.asarray(qkv_w, dtype=np.float32)
    ow = np.asarray(o_w, dtype=np.float32)
    scale = 1.0 / np.sqrt(HD)
    per_core = []
    for c in range(NCORES):
        cols = np.empty((768, H), np.float32)
        for t in range(4):                    # qk dim-tiles
            qk_sel = 0 if t < 2 else 1        # 0 = q, 1 = k
            for u in range(2):
                hl = 2 * (t % 2) + u
                hg = HPC * c + hl
                w = qkv[qk_sel * H + hg * HD: qk_sel * H + (hg + 1) * HD]
                if qk_sel == 0:
                    w = w * scale
                cols[t * 128 + u * 64: t * 128 + u * 64 + 64] = w
        for hl in range(HPC):                 # v dims
            hg = HPC * c + hl
            cols[512 + hl * 64: 512 + (hl + 1) * 64] = \
                qkv[2 * H + hg * HD: 2 * H + (hg + 1) * HD]
        wqkvT = np.ascontiguousarray(cols.T).astype(nbf16)
        woT = np.ascontiguousarray(
            ow[:, LDIM * c: LDIM * (c + 1)].T).astype(nbf16)
        per_core.append({"wqkvT": wqkvT, "woT": woT})
    return shared, per_core


_NC_CACHE = {}


def _get_nc(S=S_FULL):
    if S not in _NC_CACHE:
        _NC_CACHE[S] = build_nc(S)
    return _NC_CACHE[S]


def _run(hidden_states, qkv_w, o_w, position_ids, S=S_FULL, trace=False,
         trace_kwargs=None):
    shared, per_core = _host_prep(hidden_states, qkv_w, o_w, position_ids, S)
    in_maps = [{**shared, **per_core[c]} for c in range(NCORES)]
    nc = _get_nc(S)
    br = run_bass_kernel_spmd(
        nc, in_maps, list(range(NCORES)), trace=trace,
        **(trace_kwargs or {}))
    T = B * S
    out = np.zeros((T, H), np.float32)
    for r in br.results:
        out += r["out"].astype(np.float32)
    return out.reshape(B, S, H), br


def kernel(hidden_states, qkv_w, o_w, position_ids):
    out, _ = _run(hidden_states, qkv_w, o_w, position_ids)
    return out
